# revision 5
# baseline (speedup 1.0000x reference)
"""AttentionAggregator Trainium2 kernel (8-core SPMD, data-parallel over nodes).

Math (per node b with neighbors n):
  x_att   = lrelu_.01(x @ W_att);  neib_att = lrelu_.01(neibs @ W_att)
  e[b,n]  = lrelu_.2(x_att[b]@a_x + neib_att[b,n]@a_n)
  att     = softmax_n(e)
  agg[b]  = sum_n att[b,n] * neibs[b,n]
  out     = relu([x@W_fcx, agg@W_fcn])

Key transforms (host-side, exact):
  a_h*lrelu(z_h) summed over h is rewritten as
     sum_{seg1} relu(x . col) - sum_{seg2} relu(x . col)
  over 258 precomputed columns:
     seg1 = [.99*|a_h|*w_h : a_h>=0] + [+.01*(W@a)]
     seg2 = [.99*|a_h|*w_h : a_h<0 ] + [-.01*(W@a)]
  using lrelu(u) = .01u + .99 relu(u), a*lrelu(z)=sign(a)*lrelu(|a|z),
  k*relu(u)=relu(k*u) for k>0, and u = relu(u) - relu(-u).

On-chip per 128-node block: per-tile PE transpose of neibs (fp32, exact),
f32r scores matmul (TF32-class, logits only), relu+accumulate drains split
across ACT/DVE, softmax in a transposed [T,128] layout, attention applied
via per-tile [128,4] block-mask matmuls accumulating agg^T in PSUM (fp32),
then exact fp32 output matmuls.
"""
import warnings
warnings.filterwarnings("ignore")
import numpy as np
from contextlib import ExitStack

import concourse.bass as bass
import concourse.tile as tile
from concourse import bacc, mybir, masks
from concourse.bass_utils import run_bass_kernel_spmd

F32 = mybir.dt.float32
F32R = mybir.dt.float32r
AF = mybir.ActivationFunctionType
ALU = mybir.AluOpType
AX = mybir.AxisListType

N_CORES = 8
B_FULL, NB, D, H, O = 20000, 32, 128, 256, 128
HW6 = 2 * H // 2 + 2  # 258 score columns


def _score_weights(W_att: np.ndarray, a_half: np.ndarray):
    """Build the 258-column relu-pair score weight matrix. Returns (W6, split)."""
    pos = np.where(a_half >= 0)[0]
    neg = np.where(a_half < 0)[0]
    Wabs = W_att * np.abs(a_half)[None, :]
    w_d = (W_att @ a_half).astype(np.float64)
    seg1 = np.concatenate([0.99 * Wabs[:, pos], 0.01 * w_d[:, None]], axis=1)
    seg2 = np.concatenate([0.99 * Wabs[:, neg], -0.01 * w_d[:, None]], axis=1)
    W6 = np.concatenate([seg1, seg2], axis=1).astype(np.float32)
    return W6, seg1.shape[1]


def _blocks(bc):
    out = []
    o = 0
    while o < bc:
        f = min(128, bc - o)
        assert f * NB % 128 == 0
        out.append((o, f))
        o += f
    return out


_PROG_CACHE = {}

# test-harness knobs (harness calls kernel() with defaults: no tracing)
TRACE = False
TRACE_DIR = None
LAST_RESULTS = None


def _build_program(bc, split_n, split_x, n_cores=N_CORES, relu_blk=8):
    """Build + compile the SPMD program for bc nodes per core."""
    key = (bc, split_n, split_x, n_cores, relu_blk)
    if key in _PROG_CACHE:
        return _PROG_CACHE[key]

    nc = bacc.Bacc("TRN2", target_bir_lowering=False, debug=False,
                   num_devices=n_cores)

    x_d = nc.dram_tensor("x", [bc, D], F32R, kind="ExternalInput").ap()
    ne_d = nc.dram_tensor("ne", [bc * NB, D], F32R, kind="ExternalInput").ap()
    w6n_d = nc.dram_tensor("w6n", [D, HW6], F32, kind="ExternalInput").ap()
    w6x_d = nc.dram_tensor("w6x", [D, HW6], F32, kind="ExternalInput").ap()
    wfcx_d = nc.dram_tensor("wfcx", [D, O], F32, kind="ExternalInput").ap()
    wfcn_d = nc.dram_tensor("wfcn", [D, O], F32, kind="ExternalInput").ap()
    mask_d = nc.dram_tensor("mask", [128, 4], F32, kind="ExternalInput").ap()
    mask4_d = nc.dram_tensor("mask4", [128, 4], F32, kind="ExternalInput").ap()
    psel_d = nc.dram_tensor("psel", [128, 32], F32, kind="ExternalInput").ap()
    cful_d = nc.dram_tensor("cful", [128, HW6], F32, kind="ExternalInput").ap()
    out_d = nc.dram_tensor("out", [bc, 2 * O], F32, kind="ExternalOutput").ap()

    with tile.TileContext(nc) as tc, ExitStack() as ctx:
        consts = ctx.enter_context(tc.tile_pool(name="consts", bufs=1))
        nepool = ctx.enter_context(tc.tile_pool(name="ne", bufs=4))
        ntpool = ctx.enter_context(tc.tile_pool(name="nt", bufs=3))
        sc1 = ctx.enter_context(tc.tile_pool(name="scr_act", bufs=4))
        sc2 = ctx.enter_context(tc.tile_pool(name="scr_dve", bufs=4))
        blkpool = ctx.enter_context(tc.tile_pool(name="blk", bufs=2))
        ps_sc = ctx.enter_context(tc.tile_pool(name="ps_sc", bufs=3, space="PSUM"))
        ps_nt = ctx.enter_context(tc.tile_pool(name="ps_nt", bufs=2, space="PSUM"))
        ps_agg = ctx.enter_context(tc.tile_pool(name="ps_agg", bufs=1, space="PSUM"))
        ps_misc = ctx.enter_context(tc.tile_pool(name="ps_misc", bufs=2, space="PSUM"))

        ident = consts.tile([128, 128], F32)
        masks.make_identity(nc, ident[:])
        w6n32 = consts.tile([D, HW6], F32)
        w6x32 = consts.tile([D, HW6], F32)
        wfcx = consts.tile([D, O], F32)
        wfcn = consts.tile([D, O], F32)
        mask = consts.tile([128, 4], F32)
        mask4 = consts.tile([128, 4], F32)
        psel = consts.tile([128, 32], F32)
        cful = consts.tile([128, HW6], F32)
        for t, d in [(w6n32, w6n_d), (w6x32, w6x_d), (wfcx, wfcx_d),
                     (wfcn, wfcn_d), (mask, mask_d), (mask4, mask4_d),
                     (psel, psel_d), (cful, cful_d)]:
            nc.sync.dma_start(t[:], d)
        w6n = consts.tile([D, HW6], F32R)
        w6x = consts.tile([D, HW6], F32R)
        identr = consts.tile([128, 128], F32R)
        wfcx_r = consts.tile([D, O], F32R)
        wfcn_r = consts.tile([D, O], F32R)
        psel_r = consts.tile([128, 32], F32R)
        nc.vector.tensor_copy(w6n[:], w6n32[:])
        nc.vector.tensor_copy(w6x[:], w6x32[:])
        nc.vector.tensor_copy(identr[:], ident[:])
        nc.vector.tensor_copy(wfcx_r[:], wfcx[:])
        nc.vector.tensor_copy(wfcn_r[:], wfcn[:])
        nc.vector.tensor_copy(psel_r[:], psel[:])

        def phase1(boff, F):
            T = F * NB // 128  # score tiles in this block
            rbase = boff * NB

            ne_buf = nepool.tile([128, 32 * D], F32R, tag="ne")
            ne_v = ne_buf[:].rearrange("p (t d) -> p t d", d=D)
            nc.sync.dma_start(
                ne_v[:, :T, :],
                ne_d[rbase: rbase + 128 * T, :].rearrange(
                    "(t p) d -> p t d", p=128))

            # ---- x side
            x_sb = blkpool.tile([128, D], F32R, tag="x")
            nc.sync.dma_start(x_sb[:F, :], x_d[boff:boff + F, :])
            xt_ps = ps_misc.tile([128, 258], F32R, tag="misc")
            nc.tensor.transpose(xt_ps[:, :F], x_sb[:F, :], identr[:F, :F])
            xtr = blkpool.tile([D, 128], F32R, tag="xtr")
            nc.vector.tensor_copy(xtr[:, :F], xt_ps[:, :F])
            xs_ps = ps_misc.tile([128, 258], F32, tag="misc")
            nc.tensor.matmul(xs_ps[:F, :], xtr[:, :F], w6x[:], start=True, stop=True)
            sxacc = blkpool.tile([128, 2], F32, tag="sxacc")
            xscr = sc1.tile([128, HW6], F32, tag="scr_a")
            nc.scalar.activation(xscr[:F, :split_x], xs_ps[:F, :split_x], AF.Relu,
                                 accum_out=sxacc[:F, 0:1])
            nc.scalar.activation(xscr[:F, split_x:HW6], xs_ps[:F, split_x:HW6],
                                 AF.Relu, accum_out=sxacc[:F, 1:2])
            sx = blkpool.tile([128, 1], F32, tag="sx")
            nc.vector.tensor_tensor(sx[:F, :], sxacc[:F, 0:1], sxacc[:F, 1:2],
                                    op=ALU.subtract)
            sx4 = blkpool.tile([128, 4], F32, tag="sx4")
            nc.vector.tensor_scalar(sx4[:F, :], mask4[:F, :], sx[:F, 0:1], None,
                                    op0=ALU.mult)
            sxg_ps = ps_misc.tile([128, 258], F32, tag="misc")
            nc.tensor.matmul(sxg_ps[:T, 0:4], psel[:F, :T], sx4[:F, :],
                             start=True, stop=True)
            sxg = blkpool.tile([32, 4], F32, tag="sxg")
            nc.vector.tensor_copy(sxg[:T, :], sxg_ps[:T, 0:4])

            # ---- per-tile: transpose, scores, relu+accum drains
            spos = blkpool.tile([128, 32], F32, tag="spos")
            sneg = blkpool.tile([128, 32], F32, tag="sneg")
            nc.gpsimd.memset(sneg[:, :T], 0.0)

            def emit_scores(t0, npair, nt_sb):
                for k in range(npair):
                    t = t0 + k
                    s_ps = ps_sc.tile([128, HW6], F32, tag="sc")
                    nc.tensor.matmul(s_ps[:], nt_sb[:, 128 * k:128 * (k + 1)],
                                     w6n[:], start=True, stop=True)
                    if t % 10 < 2:
                        scr = sc1.tile([128, HW6], F32, tag="scr_a")
                        nc.scalar.activation(scr[:, :split_n], s_ps[:, :split_n],
                                             AF.Relu, accum_out=spos[:, t:t + 1])
                        nc.scalar.activation(scr[:, split_n:HW6],
                                             s_ps[:, split_n:HW6], AF.Relu,
                                             accum_out=sneg[:, t:t + 1])
                    else:
                        scr = sc2.tile([128, HW6], F32, tag="scr_d")
                        nc.vector.scalar_tensor_tensor(
                            scr[:], s_ps[:], 0.0, cful[:],
                            op0=ALU.max, op1=ALU.mult,
                            accum_out=spos[:, t:t + 1])

            lags = []
            for t0 in range(0, T, 2):
                npair = min(2, T - t0)
                nt_ps = ps_nt.tile([128, 256], F32R, tag="nt")
                for k in range(npair):
                    t = t0 + k
                    nc.tensor.transpose(nt_ps[:, 128 * k:128 * (k + 1)],
                                        ne_v[:, t, :], identr[:])
                nt_sb = ntpool.tile([128, 256], F32R, tag="nt")
                if (t0 // 2) % 4 == 3:
                    nc.vector.tensor_copy(nt_sb[:, :128 * npair],
                                          nt_ps[:, :128 * npair])
                else:
                    nc.scalar.copy(nt_sb[:, :128 * npair], nt_ps[:, :128 * npair])
                lags.append((t0, npair, nt_sb))
                if len(lags) > 1:
                    emit_scores(*lags.pop(0))
            for l in lags:
                emit_scores(*l)

            return dict(ne_v=ne_v, xtr=xtr, T=T, F=F, boff=boff,
                        spos=spos, sneg=sneg, sxg=sxg)

        def phase1b(st):
            T, F = st["T"], st["F"]
            spos, sneg, sxg = st["spos"], st["sneg"], st["sxg"]
            # ---- softmax over neighbors in [T, 128] layout
            s_col = blkpool.tile([128, 32], F32, tag="s_col")
            nc.vector.tensor_tensor(s_col[:, :T], spos[:, :T], sneg[:, :T],
                                    op=ALU.subtract)
            snt_ps = ps_misc.tile([128, 258], F32, tag="misc")
            nc.tensor.transpose(snt_ps[:T, :128], s_col[:, :T], ident[:])
            z = blkpool.tile([32, 128], F32, tag="z")
            nc.vector.tensor_tensor(
                z[:T, :].rearrange("t (j n) -> t j n", n=32),
                snt_ps[:T, :128].rearrange("t (j n) -> t j n", n=32),
                sxg[:T, :].unsqueeze(2).broadcast_to([T, 4, 32]),
                op=ALU.add)
            zl = blkpool.tile([32, 128], F32, tag="zl")
            nc.vector.scalar_tensor_tensor(zl[:T, :], z[:T, :], 0.2, z[:T, :],
                                           op0=ALU.mult, op1=ALU.max)
            ex = blkpool.tile([32, 128], F32, tag="ex")
            nc.scalar.activation(ex[:T, :], zl[:T, :], AF.Exp)
            sums = blkpool.tile([32, 4], F32, tag="sums")
            nc.vector.tensor_reduce(
                sums[:T, :], ex[:T, :].rearrange("t (j n) -> t j n", n=32),
                axis=AX.X, op=ALU.add)
            rec = blkpool.tile([32, 4], F32, tag="rec")
            nc.vector.reciprocal(rec[:T, :], sums[:T, :])
            att = blkpool.tile([32, 128], F32, tag="att")
            nc.vector.tensor_tensor(
                att[:T, :].rearrange("t (j n) -> t j n", n=32),
                ex[:T, :].rearrange("t (j n) -> t j n", n=32),
                rec[:T, :].unsqueeze(2).broadcast_to([T, 4, 32]),
                op=ALU.mult)
            att_ps = ps_misc.tile([128, 258], F32, tag="misc")
            nc.tensor.transpose(att_ps[:, :T], att[:T, :], ident[:T, :T])
            a_all = blkpool.tile([128, 128], F32R, tag="a_all")
            nc.vector.tensor_tensor(
                a_all[:].rearrange("p (t j) -> p t j", j=4)[:, :T, :],
                mask[:].unsqueeze(1).broadcast_to([128, T, 4]),
                att_ps[:, :T].unsqueeze(2).broadcast_to([128, T, 4]),
                op=ALU.mult)
            st["a_all"] = a_all

        def phase2(st):
            ne_v, a_all, xtr = st["ne_v"], st["a_all"], st["xtr"]
            T, F, boff = st["T"], st["F"], st["boff"]
            agg_ps = ps_agg.tile([128, 128], F32, tag="agg")
            a_v = a_all[:].rearrange("p (t j) -> p t j", j=4)
            for t in range(T):
                nc.tensor.matmul(agg_ps[:, 4 * t:4 * (t + 1)], ne_v[:, t, :],
                                 a_v[:, t, :], start=True, stop=True)
            aggt = blkpool.tile([D, 128], F32R, tag="aggt")
            nc.vector.tensor_copy(aggt[:, :F], agg_ps[:, :F])

            fc_ps = ps_misc.tile([128, 258], F32, tag="misc")
            nc.tensor.matmul(fc_ps[:F, 0:O], xtr[:, :F], wfcx_r[:],
                             start=True, stop=True)
            nc.tensor.matmul(fc_ps[:F, O:2 * O], aggt[:, :F], wfcn_r[:],
                             start=True, stop=True)
            out_sb = blkpool.tile([128, 2 * O], F32, tag="out")
            nc.vector.tensor_scalar(out_sb[:F, :], fc_ps[:F, :2 * O], 0.0, None,
                                    op0=ALU.max)
            nc.sync.dma_start(out_d[boff:boff + F, :], out_sb[:F, :])

        prev = None
        for (boff, F) in _blocks(bc):
            st = phase1(boff, F)
            if prev is not None:
                phase2(prev)
            phase1b(st)
            prev = st
        phase2(prev)

    nc.compile()
    _PROG_CACHE[key] = nc
    return nc


def kernel(x, neibs, W_att, W_fcx, W_fcn, a, n_cores=N_CORES):
    x = np.asarray(x, dtype=np.float32)
    neibs = np.asarray(neibs, dtype=np.float32)
    W_att = np.asarray(W_att, dtype=np.float32)
    W_fcx = np.asarray(W_fcx, dtype=np.float32)
    W_fcn = np.asarray(W_fcn, dtype=np.float32)
    a = np.asarray(a, dtype=np.float32)

    B = x.shape[0]
    bc = B // n_cores
    a_x, a_n = a[:H, 0], a[H:, 0]
    w6x_np, split_x = _score_weights(W_att, a_x)
    w6n_np, split_n = _score_weights(W_att, a_n)
    mask_np = np.equal.outer(np.arange(128) // 32, np.arange(4)).astype(np.float32)
    mask4_np = np.equal.outer(np.arange(128) % 4, np.arange(4)).astype(np.float32)
    psel_np = np.equal.outer(np.arange(128) // 4, np.arange(32)).astype(np.float32)

    nc = _build_program(bc, split_n, split_x, n_cores)

    cvec = np.concatenate([np.ones(split_n), -np.ones(HW6 - split_n)]).astype(np.float32)
    cful_np = np.repeat(cvec[None, :], 128, axis=0)
    shared = {"w6n": w6n_np, "w6x": w6x_np, "wfcx": W_fcx, "wfcn": W_fcn,
              "mask": mask_np, "mask4": mask4_np, "psel": psel_np, "cful": cful_np}
    in_maps = []
    for c in range(n_cores):
        in_maps.append({
            "x": x[c * bc:(c + 1) * bc],
            "ne": neibs[c * bc * NB:(c + 1) * bc * NB],
            **shared,
        })
    global LAST_RESULTS
    res = run_bass_kernel_spmd(nc, in_maps, core_ids=list(range(n_cores)),
                               trace=TRACE, tmpdir=TRACE_DIR)
    LAST_RESULTS = res
    return np.concatenate([res.results[c]["out"] for c in range(n_cores)], axis=0)



# revision 26
# speedup vs baseline: 1.9184x; 1.9184x over previous
"""AttentionAggregator Trainium2 kernel (8-core SPMD, data-parallel over nodes).

Math (per node b with neighbors n):
  x_att   = lrelu_.01(x @ W_att);  neib_att = lrelu_.01(neibs @ W_att)
  e[b,n]  = lrelu_.2(x_att[b]@a_x + neib_att[b,n]@a_n)
  att     = softmax_n(e)
  agg[b]  = sum_n att[b,n] * neibs[b,n]
  out     = relu([x@W_fcx, agg@W_fcn])

Score rewrite (host-side, exact in the weights):
  sum_h a_h*lrelu(z_h) = sum_{seg1} relu(x.col) - sum_{seg2} relu(x.col)
  over 258 columns (relu-pair form), via lrelu(u)=.01u+.99relu(u),
  a*lrelu(z)=sign(a)*lrelu(|a|z), k*relu(u)=relu(k*u), u=relu(u)-relu(-u).

v2 design:
  - neibs cast to bf16 on host and laid out p-major per 128-node block so
    the natural load is fully contiguous per partition.
  - transposed neibs tiles come from the DMA xbar transpose (bf16) straight
    from HBM -> no PE transposes, no PSUM->SBUF tile copies.
  - x shipped pre-transposed bf16 from host (used for logits and fc).
  - all matmuls bf16 (FWL weight loads, 1 cycle/row): scores, agg, fc.
  - score relu+/-accumulate drains rotated across DVE/Pool/ACT (greedy
    load balance); softmax in a transposed [T,128] layout, bf16 where it
    doesn't hurt; outputs fp32.
"""
import warnings
warnings.filterwarnings("ignore")
import numpy as np
import ml_dtypes
from contextlib import ExitStack

import concourse.bass as bass
import concourse.tile as tile
from concourse import bacc, mybir, masks
from concourse.bass_utils import run_bass_kernel_spmd

F32 = mybir.dt.float32
BF16 = mybir.dt.bfloat16
AF = mybir.ActivationFunctionType
ALU = mybir.AluOpType
AX = mybir.AxisListType

N_CORES = 8
B_FULL, NB, D, H, O = 20000, 32, 128, 256, 128
HW6 = H + 2  # 258 score columns


def _score_weights(W_att: np.ndarray, a_half: np.ndarray):
    """Build the 258-column relu-pair score weight matrix. Returns (W6, split)."""
    pos = np.where(a_half >= 0)[0]
    neg = np.where(a_half < 0)[0]
    Wabs = W_att * np.abs(a_half)[None, :]
    w_d = (W_att @ a_half).astype(np.float64)
    seg1 = np.concatenate([0.99 * Wabs[:, pos], 0.01 * w_d[:, None]], axis=1)
    seg2 = np.concatenate([0.99 * Wabs[:, neg], -0.01 * w_d[:, None]], axis=1)
    W6 = np.concatenate([seg1, seg2], axis=1).astype(np.float32)
    return W6, seg1.shape[1]


def _blocks(bc):
    out = []
    o = 0
    while o < bc:
        f = min(128, bc - o)
        assert f * NB % 128 == 0
        out.append((o, f))
        o += f
    return out


_PROG_CACHE = {}

# test-harness knobs (harness calls kernel() with defaults: no tracing)
TRACE = False
TRACE_DIR = None
LAST_RESULTS = None


def _drain_engines(T):
    """Greedy per-tile drain-engine assignment balancing per-block load.

    "V": single fused drain on DVE (PSUM-capable).
    "A": two-segment Relu drain on ACT (accum+ / accum-).
    Pool has no PSUM port and no free-axis accumulate, so it only gets
    SBUF-side softmax work. Costs (ns) per drain; handicaps model each
    engine's other per-block duties.
    """
    load = {"V": 2000.0, "A": 1300.0}
    cost = {"V": 480.0, "A": 1175.0}
    plan = []
    for _ in range(T):
        e = min(load, key=lambda k: load[k] + cost[k])
        load[e] += cost[e]
        plan.append(e)
    return plan


def _build_program(bc, split_n, split_x, n_cores=N_CORES):
    """Build + compile the SPMD program for bc nodes per core."""
    key = (bc, split_n, split_x, n_cores)
    if key in _PROG_CACHE:
        return _PROG_CACHE[key]

    nc = bacc.Bacc("TRN2", target_bir_lowering=False, debug=False,
                   num_devices=n_cores)

    ne_d = nc.dram_tensor("ne", [bc * NB, D], BF16, kind="ExternalInput").ap()
    netm_d = nc.dram_tensor("netm", [bc * NB, D], BF16, kind="ExternalInput").ap()
    xt_d = nc.dram_tensor("xt", [D, bc], BF16, kind="ExternalInput").ap()
    w6n_d = nc.dram_tensor("w6n", [D, HW6], BF16, kind="ExternalInput").ap()
    w6x_d = nc.dram_tensor("w6x", [D, HW6], BF16, kind="ExternalInput").ap()
    wfcx_d = nc.dram_tensor("wfcx", [D, O], BF16, kind="ExternalInput").ap()
    wfcn_d = nc.dram_tensor("wfcn", [D, O], BF16, kind="ExternalInput").ap()
    mask_d = nc.dram_tensor("mask", [128, 4], BF16, kind="ExternalInput").ap()
    mask4_d = nc.dram_tensor("mask4", [128, 4], BF16, kind="ExternalInput").ap()
    psel_d = nc.dram_tensor("psel", [128, 32], BF16, kind="ExternalInput").ap()
    cful_d = nc.dram_tensor("cful", [128, HW6], F32, kind="ExternalInput").ap()
    cfux_d = nc.dram_tensor("cfux", [128, HW6], F32, kind="ExternalInput").ap()
    out_d = nc.dram_tensor("out", [bc, 2 * O], F32, kind="ExternalOutput").ap()

    with tile.TileContext(nc) as tc, ExitStack() as ctx:
        consts = ctx.enter_context(tc.tile_pool(name="consts", bufs=1))
        nepool = ctx.enter_context(tc.tile_pool(name="ne", bufs=3))
        ntpool = ctx.enter_context(tc.tile_pool(name="nt", bufs=3))
        xtpool = ctx.enter_context(tc.tile_pool(name="xtp", bufs=2))
        sc_v = ctx.enter_context(tc.tile_pool(name="scr_v", bufs=2))
        sc_p = ctx.enter_context(tc.tile_pool(name="scr_p", bufs=2))
        sc_a = ctx.enter_context(tc.tile_pool(name="scr_a", bufs=2))
        blkpool = ctx.enter_context(tc.tile_pool(name="blk", bufs=2))
        ps_sc = ctx.enter_context(tc.tile_pool(name="ps_sc", bufs=5, space="PSUM"))
        ps_agg = ctx.enter_context(tc.tile_pool(name="ps_agg", bufs=1, space="PSUM"))
        ps_misc = ctx.enter_context(tc.tile_pool(name="ps_misc", bufs=2, space="PSUM"))

        identf = consts.tile([128, 128], F32)
        masks.make_identity(nc, identf[:])
        ident = consts.tile([128, 128], BF16)
        nc.vector.tensor_copy(ident[:], identf[:])
        w6n = consts.tile([D, HW6], BF16)
        w6x = consts.tile([D, HW6], BF16)
        wfcx = consts.tile([D, O], BF16)
        wfcn = consts.tile([D, O], BF16)
        mask = consts.tile([128, 4], BF16)
        mask4 = consts.tile([128, 4], BF16)
        psel = consts.tile([128, 32], BF16)
        cful = consts.tile([128, HW6], F32)
        cfux = consts.tile([128, HW6], F32)
        for t, dd in [(w6n, w6n_d), (w6x, w6x_d), (wfcx, wfcx_d),
                      (wfcn, wfcn_d), (mask, mask_d), (mask4, mask4_d),
                      (psel, psel_d), (cful, cful_d), (cfux, cfux_d)]:
            nc.sync.dma_start(t[:], dd)

        def phase1(boff, F):
            T = F * NB // 128  # score tiles in this block
            rbase = boff * NB

            # natural (p-major host layout): partition p <- rows p*T..p*T+T
            ne_buf = nepool.tile([128, 32 * D], BF16, tag="ne")
            ne_v = ne_buf[:].rearrange("p (t d) -> p t d", d=D)
            nc.sync.dma_start(
                ne_v[:, :T, :],
                ne_d[rbase: rbase + 128 * T, :].rearrange(
                    "(p t) d -> p t d", t=T))

            # transposed via DMA xbar from the tile-major copy: SBUF col
            # (t*128+p) <- dram row 128t+p; out last dim (p) contiguous.
            nt_buf = ntpool.tile([128, 32 * 128], BF16, tag="nt")
            nt3 = nt_buf[:].rearrange("d (t p) -> d t p", p=128)
            nc.sync.dma_start_transpose(
                nt3[:, :T, :], netm_d[rbase: rbase + 128 * T, :])
            nt_v = nt3

            # ---- x side (xT shipped pre-transposed bf16)
            xtr = xtpool.tile([D, 128], BF16, tag="xtr")
            nc.sync.dma_start(xtr[:, :F], xt_d[:, boff:boff + F])
            xs_ps = ps_misc.tile([128, 258], F32, tag="misc")
            nc.tensor.matmul(xs_ps[:F, :], xtr[:, :F], w6x[:], start=True, stop=True)
            xscr = sc_v.tile([128, HW6], F32, tag="scr_v")
            sx = blkpool.tile([128, 1], F32, tag="sx")
            nc.vector.scalar_tensor_tensor(
                xscr[:F, :], xs_ps[:F, :], 0.0, cfux[:F, :],
                op0=ALU.max, op1=ALU.mult, accum_out=sx[:F, :])
            sx4 = blkpool.tile([128, 4], BF16, tag="sx4")
            nc.gpsimd.tensor_scalar(sx4[:F, :], mask4[:F, :], sx[:F, 0:1], None,
                                    op0=ALU.mult)
            sxg_ps = ps_misc.tile([128, 258], F32, tag="misc")
            nc.tensor.matmul(sxg_ps[:T, 0:4], psel[:F, :T], sx4[:F, :],
                             start=True, stop=True)
            sxg = blkpool.tile([32, 4], F32, tag="sxg")
            nc.vector.tensor_copy(sxg[:T, :], sxg_ps[:T, 0:4])

            # ---- per-tile scores + relu/accumulate drains
            spos = blkpool.tile([128, 32], F32, tag="spos")
            sneg = blkpool.tile([128, 32], F32, tag="sneg")
            nc.gpsimd.memset(sneg[:, :T], 0.0)

            plan = _drain_engines(T)
            for t in range(T):
                s_ps = ps_sc.tile([128, HW6], F32, tag="sc")
                nc.tensor.matmul(s_ps[:], nt_v[:, t, :], w6n[:],
                                 start=True, stop=True)
                if plan[t] == "V":
                    scr = sc_v.tile([128, HW6], F32, tag="scr_v")
                    nc.vector.scalar_tensor_tensor(
                        scr[:], s_ps[:], 0.0, cful[:],
                        op0=ALU.max, op1=ALU.mult,
                        accum_out=spos[:, t:t + 1])
                else:
                    scr = sc_a.tile([128, HW6], BF16, tag="scr_a")
                    nc.scalar.activation(scr[:, :split_n], s_ps[:, :split_n],
                                         AF.Relu, accum_out=spos[:, t:t + 1])
                    nc.scalar.activation(scr[:, split_n:HW6],
                                         s_ps[:, split_n:HW6], AF.Relu,
                                         accum_out=sneg[:, t:t + 1])

            return dict(ne_v=ne_v, xtr=xtr, T=T, F=F, boff=boff,
                        spos=spos, sneg=sneg, sxg=sxg)

        def phase1b(st):
            T, F = st["T"], st["F"]
            spos, sneg, sxg = st["spos"], st["sneg"], st["sxg"]
            # ---- softmax over neighbors in [T, 128] layout
            s_col = blkpool.tile([128, 32], BF16, tag="s_col")
            nc.gpsimd.tensor_tensor(s_col[:, :T], spos[:, :T], sneg[:, :T],
                                    op=ALU.subtract)
            snt_ps = ps_misc.tile([128, 258], BF16, tag="misc")
            nc.tensor.transpose(snt_ps[:T, :128], s_col[:, :T], ident[:])
            z = blkpool.tile([32, 128], F32, tag="z")
            nc.vector.tensor_tensor(
                z[:T, :].rearrange("t (j n) -> t j n", n=32),
                snt_ps[:T, :128].rearrange("t (j n) -> t j n", n=32),
                sxg[:T, :].unsqueeze(2).broadcast_to([T, 4, 32]),
                op=ALU.add)
            zl = blkpool.tile([32, 128], F32, tag="zl")
            nc.vector.scalar_tensor_tensor(zl[:T, :], z[:T, :], 0.2, z[:T, :],
                                           op0=ALU.mult, op1=ALU.max)
            ex = blkpool.tile([32, 128], F32, tag="ex")
            nc.scalar.activation(ex[:T, :], zl[:T, :], AF.Exp)
            sums = blkpool.tile([32, 4], F32, tag="sums")
            nc.vector.tensor_reduce(
                sums[:T, :], ex[:T, :].rearrange("t (j n) -> t j n", n=32),
                axis=AX.X, op=ALU.add)
            rec = blkpool.tile([32, 4], F32, tag="rec")
            nc.vector.reciprocal(rec[:T, :], sums[:T, :])
            att = blkpool.tile([32, 128], BF16, tag="att")
            nc.gpsimd.tensor_tensor(
                att[:T, :].rearrange("t (j n) -> t j n", n=32),
                ex[:T, :].rearrange("t (j n) -> t j n", n=32),
                rec[:T, :].unsqueeze(2).broadcast_to([T, 4, 32]),
                op=ALU.mult)
            att_ps = ps_misc.tile([128, 258], BF16, tag="misc")
            nc.tensor.transpose(att_ps[:, :T], att[:T, :], ident[:T, :T])
            a_all = blkpool.tile([128, 128], BF16, tag="a_all")
            nc.vector.tensor_tensor(
                a_all[:].rearrange("p (t j) -> p t j", j=4)[:, :T, :],
                mask[:].unsqueeze(1).broadcast_to([128, T, 4]),
                att_ps[:, :T].unsqueeze(2).broadcast_to([128, T, 4]),
                op=ALU.mult)
            st["a_all"] = a_all

        def phase2(st):
            ne_v, a_all, xtr = st["ne_v"], st["a_all"], st["xtr"]
            T, F, boff = st["T"], st["F"], st["boff"]
            agg_ps = ps_agg.tile([128, 128], F32, tag="agg")
            a_v = a_all[:].rearrange("p (t j) -> p t j", j=4)
            for t in range(T):
                nc.tensor.matmul(agg_ps[:, 4 * t:4 * (t + 1)], ne_v[:, t, :],
                                 a_v[:, t, :], start=True, stop=True)
            aggt = blkpool.tile([D, 128], BF16, tag="aggt")
            nc.scalar.copy(aggt[:, :F], agg_ps[:, :F])

            fc_ps = ps_misc.tile([128, 258], F32, tag="misc")
            nc.tensor.matmul(fc_ps[:F, 0:O], xtr[:, :F], wfcx[:],
                             start=True, stop=True)
            nc.tensor.matmul(fc_ps[:F, O:2 * O], aggt[:, :F], wfcn[:],
                             start=True, stop=True)
            out_sb = blkpool.tile([128, 2 * O], F32, tag="out")
            nc.scalar.activation(out_sb[:F, :], fc_ps[:F, :2 * O], AF.Relu)
            nc.sync.dma_start(out_d[boff:boff + F, :], out_sb[:F, :])

        prev = None
        for (boff, F) in _blocks(bc):
            st = phase1(boff, F)
            if prev is not None:
                phase2(prev)
            phase1b(st)
            prev = st
        phase2(prev)

    nc.compile()
    _PROG_CACHE[key] = nc
    return nc


def _permute_pmajor(ne_c: np.ndarray, bc: int) -> np.ndarray:
    """Per 128-node block, reorder rows tile-major -> partition-major."""
    chunks = []
    r = 0
    for (boff, F) in _blocks(bc):
        T = F * NB // 128
        blk = ne_c[r:r + 128 * T]  # rows ordered (t, p)
        chunks.append(blk.reshape(T, 128, D).transpose(1, 0, 2).reshape(-1, D))
        r += 128 * T
    return np.concatenate(chunks, axis=0)


def kernel(x, neibs, W_att, W_fcx, W_fcn, a, n_cores=N_CORES):
    x = np.asarray(x, dtype=np.float32)
    neibs = np.asarray(neibs, dtype=np.float32)
    W_att = np.asarray(W_att, dtype=np.float32)
    W_fcx = np.asarray(W_fcx, dtype=np.float32)
    W_fcn = np.asarray(W_fcn, dtype=np.float32)
    a = np.asarray(a, dtype=np.float32)

    B = x.shape[0]
    bc = B // n_cores
    a_x, a_n = a[:H, 0], a[H:, 0]
    w6x_np, split_x = _score_weights(W_att, a_x)
    w6n_np, split_n = _score_weights(W_att, a_n)
    mask_np = np.equal.outer(np.arange(128) // 32, np.arange(4))
    mask4_np = np.equal.outer(np.arange(128) % 4, np.arange(4))
    psel_np = np.equal.outer(np.arange(128) // 4, np.arange(32))

    nc = _build_program(bc, split_n, split_x, n_cores)

    bf = ml_dtypes.bfloat16
    cvec = np.concatenate([np.ones(split_n), -np.ones(HW6 - split_n)]).astype(np.float32)
    cful_np = np.repeat(cvec[None, :], 128, axis=0)
    cvex = np.concatenate([np.ones(split_x), -np.ones(HW6 - split_x)]).astype(np.float32)
    cfux_np = np.repeat(cvex[None, :], 128, axis=0)
    shared = {"w6n": w6n_np.astype(bf), "w6x": w6x_np.astype(bf),
              "wfcx": W_fcx.astype(bf), "wfcn": W_fcn.astype(bf),
              "mask": mask_np.astype(bf), "mask4": mask4_np.astype(bf),
              "psel": psel_np.astype(bf), "cful": cful_np, "cfux": cfux_np}
    in_maps = []
    for c in range(n_cores):
        ne_c = neibs[c * bc * NB:(c + 1) * bc * NB].astype(bf)
        in_maps.append({
            "ne": _permute_pmajor(ne_c, bc),
            "netm": ne_c,
            "xt": np.ascontiguousarray(x[c * bc:(c + 1) * bc].T).astype(bf),
            **shared,
        })
    global LAST_RESULTS
    res = run_bass_kernel_spmd(nc, in_maps, core_ids=list(range(n_cores)),
                               trace=TRACE, tmpdir=TRACE_DIR)
    LAST_RESULTS = res
    return np.concatenate([res.results[c]["out"] for c in range(n_cores)], axis=0)
